# revision 7
# baseline (speedup 1.0000x reference)
"""NNUE evaluation kernel for Trainium2 (8 NeuronCores, data-parallel batch).

reference math:
    wh = clip(white @ W_ft.T, 0, 1)        # [B, 256]
    bh = clip(black @ W_ft.T, 0, 1)        # [B, 256]
    x  = concat(wh, bh)                    # [B, 512]
    x  = relu(x @ W1.T + b1); x = relu(x @ W2.T + b2)
    ev = (x @ W3.T + b3) * stm[:, None]    # [B, 1]

Strategy: shard B=4096 across 8 cores (512 rows each), data-parallel, no
collectives. Two co-designed tricks get ~2.1x over the fp32 GEMM:

1. HBM: the kernel streams ~105 MB/core instead of ~210 MB by writing
   each fp32 operand as a SPLIT fp8 pair on the host: hi = e4m3(x),
   lo = e5m2(x - hi) (e5m2's wide exponent covers the tiny residuals
   that e4m3 would flush to zero). Same bytes as fp16, but ~11-bit
   effective mantissa; measured end-to-end rel err ~4e-3 vs the 2e-2
   budget.

2. PE: fp8 enables MatmulPerfMode.DoubleRow (2 k-rows per partition per
   cycle, 2x MAC rate). The split GEMM needs 3 DoubleRow passes
   (hi@Whi + hi@Wlo + lo@Whi; the lo@lo term is negligible and dropped),
   i.e. 3/4 of the PE cycles of the fp16 version, ~205 us vs ~277 us.
   All 3 passes accumulate into the same 4 PSUM banks (out.T [h, b] per
   side and h-tile) across all 160 k-pair-tiles, so the clip + tiny MLP
   tail is unchanged from the fp16 version.

The host pre-transposes all operands into k-major partition-first layout
(DMA reads 4-20 KB contiguous per partition per slab; all 16 DMA engines
sit at line rate ~400 GB/s with features on the sync queue and weights
on the scalar queue). Small first slabs let the PE start ~4 us into the
stream; small last slabs leave it almost nothing to chew after the last
byte lands. The clip fuses into the PSUM evacuation and the tiny MLP
runs in transposed [features, batch] layout (fp16) to the end.

This walrus build rejects instructions with >1 sync wait, so a post-pass
(_split_multi_waits) redistributes Tile-emitted waits onto single-wait
no-ops.
"""

import sys
import types

import numpy as np


def _inject_ntff_hook():
    """Register the axon NTFF profile hook if this image's antenv lacks it."""
    try:
        import antenv.axon_hooks  # noqa: F401
        return
    except ImportError:
        pass
    try:
        import trn_agent_boot.trn_boot as tb
        hook = tb._ntff_profile_via_ctypes("/opt/axon/libaxon_pjrt.so")
    except Exception:
        hook = None
    mod = types.ModuleType("antenv.axon_hooks")
    mod.get_axon_ntff_profile_hook = lambda: hook
    mod.set_axon_ntff_profile_hook = lambda h: None
    sys.modules["antenv.axon_hooks"] = mod


_inject_ntff_hook()

import concourse.bass as bass
import concourse.mybir as mybir
from concourse.tile import TileContext

N_CORES = 8
B = 4096
BS = B // N_CORES          # 512 batch rows per core
IN = 40960                 # feature count (contraction dim)
H = 256                    # hidden per perspective
NKP = IN // 256            # 160 k-pair-tiles (256 contraction rows each)
# k-pair-tiles per DMA slab. Small first slabs so the PE starts early;
# small last slabs so the PE has little left once the last byte lands.
SLABS = [2, 2] + [4] * 38 + [2, 1, 1]
assert sum(SLABS) == NKP

F32 = mybir.dt.float32
F16 = mybir.dt.float16
E4 = mybir.dt.float8e4
E5 = mybir.dt.float8e5
DR = mybir.MatmulPerfMode.DoubleRow


def _split_multi_waits(nc: bass.Bass) -> None:
    """This walrus build rejects instructions carrying more than one sync
    wait. Split any such instruction: emit single-wait no-ops on the same
    engine immediately before it (same engine stream => same semantics)."""
    for f in nc.m.functions:
        for bb in f.blocks:
            new_insts = []
            changed = False
            for inst in bb.instructions:
                si = inst.sync_info
                waits = list(si.on_wait) if si is not None and si.on_wait else []
                if len(waits) > 1:
                    changed = True
                    for i, w in enumerate(waits[:-1]):
                        nop = mybir.InstNoOp(
                            name=f"{inst.name}-sw{i}", ins=[], outs=[]
                        )
                        nop.engine = inst.engine
                        nop.sync_info = mybir.SyncInfo(on_wait=[w], on_update=[])
                        nc.register_instruction(nop)
                        new_insts.append(nop)
                    inst.sync_info = mybir.SyncInfo(
                        on_wait=[waits[-1]],
                        on_update=list(si.on_update) if si.on_update else [],
                    )
                new_insts.append(inst)
            if changed:
                bb.instructions = new_insts


def build_kernel() -> bass.Bass:
    nc = bass.Bass()

    # k-major pair layout, partition-first: t[p, kp, side, i, b] holds
    # feature[b, kp*256 + i*128 + p] for side in (white, black); weights
    # w*[p, kp, i, h] = W_ft[h, kp*256 + i*128 + p]. hi = e4m3(x),
    # lo = e5m2(x - hi).
    fhi = nc.dram_tensor("feat_hi", [128, NKP, 2, 2, BS], E4, kind="ExternalInput")
    flo = nc.dram_tensor("feat_lo", [128, NKP, 2, 2, BS], E5, kind="ExternalInput")
    whi = nc.dram_tensor("w_hi", [128, NKP, 2, H], E4, kind="ExternalInput")
    wlo = nc.dram_tensor("w_lo", [128, NKP, 2, H], E5, kind="ExternalInput")
    w1Ts = nc.dram_tensor("W1Ts", [128, 128], F16, kind="ExternalInput")
    b1 = nc.dram_tensor("b1", [32, 1], F32, kind="ExternalInput")
    w2T = nc.dram_tensor("W2T", [32, 32], F16, kind="ExternalInput")
    b2 = nc.dram_tensor("b2", [32, 1], F32, kind="ExternalInput")
    w3T = nc.dram_tensor("W3T", [32, 1], F16, kind="ExternalInput")
    b3 = nc.dram_tensor("b3", [1, 1], F32, kind="ExternalInput")
    stm = nc.dram_tensor("side_to_move", [1, BS], F32, kind="ExternalInput")
    out = nc.dram_tensor("evaluation", [1, BS], F32, kind="ExternalOutput")

    with TileContext(nc) as tc:
        with (
            tc.tile_pool(name="ot_psum", bufs=1, space="PSUM") as ot_pool,
            tc.tile_pool(name="mlp", bufs=1) as mlp,
        ):
            # out.T accumulators: [h-tile 128, b 512] x (2 sides x 2 h-tiles)
            ot = [
                ot_pool.tile([128, BS], F32, tag=f"ot{i}", name=f"ot{i}")
                for i in range(4)
            ]

            # ---- MLP weight prep: tiny DMAs, issued up-front on the
            # scalar queue so the tail never waits ----
            w1t = mlp.tile([128, 4, 32], F16)
            nc.scalar.dma_start(out=w1t[:], in_=w1Ts[:, :])
            w2t = mlp.tile([32, 32], F16)
            nc.scalar.dma_start(out=w2t[:], in_=w2T[:, :])
            w3t = mlp.tile([32, 1], F16)
            nc.scalar.dma_start(out=w3t[:], in_=w3T[:, :])
            b1_sb = mlp.tile([32, 1], F32)
            nc.scalar.dma_start(out=b1_sb[:], in_=b1[:, :])
            b2_sb = mlp.tile([32, 1], F32)
            nc.scalar.dma_start(out=b2_sb[:], in_=b2[:, :])
            b3_sb = mlp.tile([1, 1], F32)
            nc.scalar.dma_start(out=b3_sb[:], in_=b3[:, :])
            stm_sb = mlp.tile([1, BS], F32)
            nc.scalar.dma_start(out=stm_sb[:], in_=stm[:, :])

            # ---- main loop: split-fp8 feature-transformer GEMMs ----
            with (
                tc.tile_pool(name="rampf", bufs=1) as rampf_pool,
                tc.tile_pool(name="rampw", bufs=1) as rampw_pool,
                tc.tile_pool(name="fhi", bufs=6) as fhi_pool,
                tc.tile_pool(name="flo", bufs=6) as flo_pool,
                tc.tile_pool(name="whi", bufs=6) as whi_pool,
                tc.tile_pool(name="wlo", bufs=6) as wlo_pool,
            ):
                kp_base = 0
                for si, sk in enumerate(SLABS):
                    ramp = sk < 4
                    if ramp:
                        th = rampf_pool.tile([128, sk, 2, 2, BS], E4, tag=f"fh{si}")
                        tl = rampf_pool.tile([128, sk, 2, 2, BS], E5, tag=f"fl{si}")
                        wh = rampw_pool.tile([128, sk, 2, H], E4, tag=f"wh{si}")
                        wl = rampw_pool.tile([128, sk, 2, H], E5, tag=f"wl{si}")
                    else:
                        th = fhi_pool.tile([128, sk, 2, 2, BS], E4, tag="fhi")
                        tl = flo_pool.tile([128, sk, 2, 2, BS], E5, tag="flo")
                        wh = whi_pool.tile([128, sk, 2, H], E4, tag="whi")
                        wl = wlo_pool.tile([128, sk, 2, H], E5, tag="wlo")
                    ksl = slice(kp_base, kp_base + sk)
                    nc.sync.dma_start(out=th[:], in_=fhi[:, ksl])
                    nc.sync.dma_start(out=tl[:], in_=flo[:, ksl])
                    nc.scalar.dma_start(out=wh[:], in_=whi[:, ksl])
                    nc.scalar.dma_start(out=wl[:], in_=wlo[:, ksl])
                    for kp in range(sk):
                        g = kp_base + kp
                        first = g == 0
                        last = g == NKP - 1
                        # 3 passes: hi@Whi, hi@Wlo, lo@Whi — rotate PSUM
                        # banks every matmul ((side, h) inner).
                        for pi, (wt, ft) in enumerate(
                            ((wh, th), (wl, th), (wh, tl))
                        ):
                            for side in range(2):
                                for h in range(2):
                                    nc.tensor.matmul(
                                        ot[side * 2 + h],
                                        wt[:, kp, :, h * 128:(h + 1) * 128],
                                        ft[:, kp, side],
                                        start=first and pi == 0,
                                        stop=last and pi == 2,
                                        perf_mode=DR,
                                    )
                    kp_base += sk

            # ---- clip + MLP (transposed layout throughout) ----
            with tc.tile_pool(name="mlp2_psum", bufs=1, space="PSUM") as mpp2:
                xt = []
                for i in range(4):
                    t = mlp.tile([128, BS], F16, tag=f"xt{i}")
                    nc.vector.tensor_scalar(
                        out=t[:], in0=ot[i][:], scalar1=0.0, scalar2=1.0,
                        op0=mybir.AluOpType.max, op1=mybir.AluOpType.min,
                    )
                    xt.append(t)

                h1p = mpp2.tile([32, BS], F32, tag="h1")
                for kt in range(4):
                    nc.tensor.matmul(
                        h1p, w1t[:, kt, :], xt[kt][:],
                        start=kt == 0, stop=kt == 3,
                    )
                h1 = mlp.tile([32, BS], F16)
                nc.vector.tensor_scalar(
                    out=h1[:], in0=h1p[:], scalar1=b1_sb[:, :], scalar2=0.0,
                    op0=mybir.AluOpType.add, op1=mybir.AluOpType.max,
                )

                h2p = mpp2.tile([32, BS], F32, tag="h2")
                nc.tensor.matmul(
                    h2p, w2t[:], h1[:], start=True, stop=True
                )
                h2 = mlp.tile([32, BS], F16)
                nc.vector.tensor_scalar(
                    out=h2[:], in0=h2p[:], scalar1=b2_sb[:, :], scalar2=0.0,
                    op0=mybir.AluOpType.add, op1=mybir.AluOpType.max,
                )

                evp = mpp2.tile([1, BS], F32, tag="ev")
                nc.tensor.matmul(
                    evp, w3t[:], h2[:], start=True, stop=True
                )
                ev = mlp.tile([1, BS], F32)
                nc.vector.tensor_scalar(
                    out=ev[:], in0=evp[:], scalar1=b3_sb[:, :], scalar2=None,
                    op0=mybir.AluOpType.add,
                )
                evs = mlp.tile([1, BS], F32)
                nc.vector.tensor_mul(out=evs[:], in0=ev[:], in1=stm_sb[:])
                nc.sync.dma_start(out=out[:, :], in_=evs[:])

    _split_multi_waits(nc)
    return nc


_NC_CACHE: dict = {}


def _get_nc(**_ignored) -> bass.Bass:
    if "nc" not in _NC_CACHE:
        _NC_CACHE["nc"] = build_kernel()
    return _NC_CACHE["nc"]


def _np_dt(dt):
    return np.dtype(mybir.dt.np(dt))


def _split8(a32: np.ndarray):
    """fp32 -> (e4m3 hi, e5m2 lo) split arrays (numpy, RNE rounding)."""
    hi = a32.astype(_np_dt(E4))
    lo = (a32 - hi.astype(np.float32)).astype(_np_dt(E5))
    return hi, lo


def _kpair(a8: np.ndarray, ncols: int) -> np.ndarray:
    """[ncols, IN] fp8 -> [128, NKP, 2, ncols], t[p,kp,i,c] = a[c, kp*256+i*128+p]."""
    return np.ascontiguousarray(
        a8.reshape(ncols, NKP, 2, 128).transpose(3, 1, 2, 0)
    )


def make_in_maps(inputs: dict) -> list:
    """Shard full inputs into per-core maps (split + transpose on host)."""
    wf = np.asarray(inputs["white_features"], dtype=np.float32)
    bf = np.asarray(inputs["black_features"], dtype=np.float32)
    stm = np.ascontiguousarray(inputs["side_to_move"], dtype=np.float32)
    whi8, wlo8 = _split8(np.asarray(inputs["W_ft"], dtype=np.float32))
    whi_k = _kpair(whi8, H)
    wlo_k = _kpair(wlo8, H)
    w1T = np.asarray(inputs["W1"], dtype=np.float32).T  # [512, 32]
    w1Ts = np.ascontiguousarray(
        w1T.reshape(4, 128, 32).transpose(1, 0, 2)
    ).reshape(128, 128).astype(np.float16)
    w2T = np.ascontiguousarray(
        np.asarray(inputs["W2"], dtype=np.float32).T).astype(np.float16)
    w3T = np.ascontiguousarray(
        np.asarray(inputs["W3"], dtype=np.float32).T).astype(np.float16)
    b1 = np.ascontiguousarray(inputs["b1"], dtype=np.float32).reshape(32, 1)
    b2 = np.ascontiguousarray(inputs["b2"], dtype=np.float32).reshape(32, 1)
    b3 = np.ascontiguousarray(inputs["b3"], dtype=np.float32).reshape(1, 1)
    maps = []
    for c in range(N_CORES):
        sl = slice(c * BS, (c + 1) * BS)
        f_hi = np.empty((128, NKP, 2, 2, BS), dtype=_np_dt(E4))
        f_lo = np.empty((128, NKP, 2, 2, BS), dtype=_np_dt(E5))
        for side, feat in enumerate((wf, bf)):
            hi8, lo8 = _split8(feat[sl])
            f_hi[:, :, side] = _kpair(hi8, BS)
            f_lo[:, :, side] = _kpair(lo8, BS)
        maps.append({
            "feat_hi": f_hi,
            "feat_lo": f_lo,
            "w_hi": whi_k,
            "w_lo": wlo_k,
            "side_to_move": stm[sl].reshape(1, BS),
            "W1Ts": w1Ts,
            "b1": b1,
            "W2T": w2T,
            "b2": b2,
            "W3T": w3T,
            "b3": b3,
        })
    return maps


def run(inputs: dict, trace: bool = False, **_ignored):
    """Run on all 8 cores; returns (full_output [4096,1] fp32, BassKernelResults)."""
    from concourse.bass_utils import run_bass_kernel_spmd

    nc = _get_nc()
    res = run_bass_kernel_spmd(
        nc, make_in_maps(inputs), core_ids=list(range(N_CORES)), trace=trace
    )
    full = np.concatenate(
        [res.results[c]["evaluation"].reshape(BS, 1) for c in range(N_CORES)],
        axis=0,
    ).astype(np.float32)
    return full, res


def kernel(**inputs) -> np.ndarray:
    return run(inputs, trace=False)[0]


if __name__ == "__main__":
    rng = np.random.default_rng(0)
    ins = {
        "white_features": rng.random((B, IN), dtype=np.float32),
        "black_features": rng.random((B, IN), dtype=np.float32),
        "side_to_move": np.ones((B,), dtype=np.float32),
        "W_ft": (0.1 * rng.standard_normal((H, IN))).astype(np.float32),
        "W1": (0.06 * rng.standard_normal((32, 2 * H))).astype(np.float32),
        "b1": np.zeros(32, np.float32),
        "W2": (0.17 * rng.standard_normal((32, 32))).astype(np.float32),
        "b2": np.zeros(32, np.float32),
        "W3": (0.24 * rng.standard_normal((1, 32))).astype(np.float32),
        "b3": np.zeros(1, np.float32),
    }
    out = kernel(**ins)
    # host reference
    whr = np.clip(ins["white_features"] @ ins["W_ft"].T, 0, 1)
    bhr = np.clip(ins["black_features"] @ ins["W_ft"].T, 0, 1)
    x = np.concatenate([whr, bhr], axis=1)
    x = np.maximum(x @ ins["W1"].T + ins["b1"], 0)
    x = np.maximum(x @ ins["W2"].T + ins["b2"], 0)
    ref = (x @ ins["W3"].T + ins["b3"]) * ins["side_to_move"][:, None]
    rel = np.linalg.norm(out - ref) / np.linalg.norm(ref)
    print("rel err:", rel)


# revision 10
# speedup vs baseline: 1.3258x; 1.3258x over previous
"""NNUE evaluation kernel for Trainium2 (8 NeuronCores, data-parallel batch).

reference math:
    wh = clip(white @ W_ft.T, 0, 1)        # [B, 256]
    bh = clip(black @ W_ft.T, 0, 1)        # [B, 256]
    x  = concat(wh, bh)                    # [B, 512]
    x  = relu(x @ W1.T + b1); x = relu(x @ W2.T + b2)
    ev = (x @ W3.T + b3) * stm[:, None]    # [B, 1]

Strategy: shard B=4096 across 8 cores (512 rows each), data-parallel, no
collectives. The kernel is HBM-bound, so everything on the feature-GEMM
path is cast to fp16 (e5m10) on the host: features are uniform [0,1] and
W_ft is N(0, 0.1), both comfortably inside fp16 range, and the PE
multiplies bf16/fp16 at the full 1 row/cycle rate with fp32 PSUM
accumulation (simulated end-to-end rel err ~7e-4 vs 2e-2 budget). That
halves the per-core HBM traffic from ~210 MB to ~105 MB (~276 us at the
~380 GB/s 16-engine DMA ceiling measured on this part).

The host pre-transposes everything into k-major partition-first layout
and PACKS white, black and W_ft.T into a single [128, 320, 1280] fp16
array per core (per k-tile: 512 white cols | 512 black cols | 256 W_ftT
cols). The kernel then needs NO on-chip transposes (the fp32 baseline
burned ~300 us of PE+DVE on matmul-with-identity transposes) and the
whole 105 MB streams as ONE run of large DMAs on one queue — 40 KB
contiguous per partition per slab — which keeps all 16 DMA engines at
line rate with no inter-queue arbitration. The first slabs are small
(4/4/8 k-tiles vs 16) so the PE starts ~5 us after launch instead of
waiting for a full 5 MB slab. out.T [h, b] accumulates in 4 PSUM banks
across all 320 k-tiles, the clip fuses into PSUM evacuation, and the
tiny MLP runs in transposed [features, batch] layout to the end.

This walrus build rejects instructions with >1 sync wait, so a post-pass
(_split_multi_waits) redistributes Tile-emitted waits onto single-wait
no-ops.
"""

import sys
import types

import numpy as np


def _inject_ntff_hook():
    """Register the axon NTFF profile hook if this image's antenv lacks it."""
    try:
        import antenv.axon_hooks  # noqa: F401
        return
    except ImportError:
        pass
    try:
        import trn_agent_boot.trn_boot as tb
        hook = tb._ntff_profile_via_ctypes("/opt/axon/libaxon_pjrt.so")
    except Exception:
        hook = None
    mod = types.ModuleType("antenv.axon_hooks")
    mod.get_axon_ntff_profile_hook = lambda: hook
    mod.set_axon_ntff_profile_hook = lambda h: None
    sys.modules["antenv.axon_hooks"] = mod


_inject_ntff_hook()

import concourse.bass as bass
import concourse.mybir as mybir
from concourse.tile import TileContext

N_CORES = 8
B = 4096
BS = B // N_CORES          # 512 batch rows per core
IN = 40960                 # feature count (contraction dim)
H = 256                    # hidden per perspective
NKT = IN // 128            # 320 k-tiles of 128
PK = 2 * BS + H            # packed columns per k-tile: white | black | W_ftT
# k-tiles per DMA slab. Small first slabs so the PE starts ~4 us after
# the stream opens instead of waiting on a full slab; small last slabs so
# the PE (which trails the DMA stream by ~one slab) has almost nothing
# left to chew once the last byte lands.
SLABS = [2, 2, 4] + [8] * 38 + [4, 2, 2]
assert sum(SLABS) == NKT

F32 = mybir.dt.float32
F16 = mybir.dt.float16


def _split_multi_waits(nc: bass.Bass) -> None:
    """This walrus build rejects instructions carrying more than one sync
    wait. Split any such instruction: emit single-wait no-ops on the same
    engine immediately before it (same engine stream => same semantics)."""
    for f in nc.m.functions:
        for bb in f.blocks:
            new_insts = []
            changed = False
            for inst in bb.instructions:
                si = inst.sync_info
                waits = list(si.on_wait) if si is not None and si.on_wait else []
                if len(waits) > 1:
                    changed = True
                    for i, w in enumerate(waits[:-1]):
                        nop = mybir.InstNoOp(
                            name=f"{inst.name}-sw{i}", ins=[], outs=[]
                        )
                        nop.engine = inst.engine
                        nop.sync_info = mybir.SyncInfo(on_wait=[w], on_update=[])
                        nc.register_instruction(nop)
                        new_insts.append(nop)
                    inst.sync_info = mybir.SyncInfo(
                        on_wait=[waits[-1]],
                        on_update=list(si.on_update) if si.on_update else [],
                    )
                new_insts.append(inst)
            if changed:
                bb.instructions = new_insts


def build_kernel() -> bass.Bass:
    nc = bass.Bass()

    # packed[p, kt, :] = [white[b, kt*128+p] for b in 512] ++ [black ...]
    #                    ++ [W_ft[h, kt*128+p] for h in 256]
    packed = nc.dram_tensor("packed", [128, NKT, PK], F16, kind="ExternalInput")
    w1Ts = nc.dram_tensor("W1Ts", [128, 128], F16, kind="ExternalInput")
    b1 = nc.dram_tensor("b1", [32, 1], F32, kind="ExternalInput")
    w2T = nc.dram_tensor("W2T", [32, 32], F16, kind="ExternalInput")
    b2 = nc.dram_tensor("b2", [32, 1], F32, kind="ExternalInput")
    w3T = nc.dram_tensor("W3T", [32, 1], F16, kind="ExternalInput")
    b3 = nc.dram_tensor("b3", [1, 1], F32, kind="ExternalInput")
    stm = nc.dram_tensor("side_to_move", [1, BS], F32, kind="ExternalInput")
    out = nc.dram_tensor("evaluation", [1, BS], F32, kind="ExternalOutput")

    with TileContext(nc) as tc:
        with (
            tc.tile_pool(name="ot_psum", bufs=1, space="PSUM") as ot_pool,
            tc.tile_pool(name="mlp", bufs=1) as mlp,
        ):
            # out.T accumulators: [h-tile 128, b 512] x (2 sides x 2 h-tiles)
            ot = [
                ot_pool.tile([128, BS], F32, tag=f"ot{i}", name=f"ot{i}")
                for i in range(4)
            ]

            # ---- MLP weight prep: tiny DMAs, issued up-front on the
            # (otherwise idle) scalar queue so the tail never waits ----
            w1t = mlp.tile([128, 4, 32], F16)
            nc.scalar.dma_start(out=w1t[:], in_=w1Ts[:, :])
            w2t = mlp.tile([32, 32], F16)
            nc.scalar.dma_start(out=w2t[:], in_=w2T[:, :])
            w3t = mlp.tile([32, 1], F16)
            nc.scalar.dma_start(out=w3t[:], in_=w3T[:, :])
            b1_sb = mlp.tile([32, 1], F32)
            nc.scalar.dma_start(out=b1_sb[:], in_=b1[:, :])
            b2_sb = mlp.tile([32, 1], F32)
            nc.scalar.dma_start(out=b2_sb[:], in_=b2[:, :])
            b3_sb = mlp.tile([1, 1], F32)
            nc.scalar.dma_start(out=b3_sb[:], in_=b3[:, :])
            stm_sb = mlp.tile([1, BS], F32)
            nc.scalar.dma_start(out=stm_sb[:], in_=stm[:, :])

            # ---- main loop: feature-transformer GEMMs ----
            with (
                tc.tile_pool(name="ramp", bufs=1) as ramp_pool,
                tc.tile_pool(name="slab", bufs=7) as slab_pool,
            ):
                kt_base = 0
                for si, sk in enumerate(SLABS):
                    if sk < 8:
                        t = ramp_pool.tile([128, sk, PK], F16, tag=f"r{si}")
                    else:
                        t = slab_pool.tile([128, sk, PK], F16, tag="slab")
                    nc.sync.dma_start(
                        out=t[:], in_=packed[:, kt_base:kt_base + sk, :]
                    )
                    for kt in range(sk):
                        g = kt_base + kt
                        first = g == 0
                        last = g == NKT - 1
                        for side in range(2):
                            for h in range(2):
                                nc.tensor.matmul(
                                    ot[side * 2 + h],
                                    t[:, kt, 2 * BS + h * 128:2 * BS + (h + 1) * 128],
                                    t[:, kt, side * BS:(side + 1) * BS],
                                    start=first,
                                    stop=last,
                                )
                    kt_base += sk

            # ---- clip + MLP (transposed layout throughout) ----
            with tc.tile_pool(name="mlp2_psum", bufs=1, space="PSUM") as mpp2:
                xt = []
                for i in range(4):
                    t = mlp.tile([128, BS], F16, tag=f"xt{i}")
                    nc.vector.tensor_scalar(
                        out=t[:], in0=ot[i][:], scalar1=0.0, scalar2=1.0,
                        op0=mybir.AluOpType.max, op1=mybir.AluOpType.min,
                    )
                    xt.append(t)

                h1p = mpp2.tile([32, BS], F32, tag="h1")
                for kt in range(4):
                    nc.tensor.matmul(
                        h1p, w1t[:, kt, :], xt[kt][:],
                        start=kt == 0, stop=kt == 3,
                    )
                h1 = mlp.tile([32, BS], F16)
                nc.vector.tensor_scalar(
                    out=h1[:], in0=h1p[:], scalar1=b1_sb[:, :], scalar2=0.0,
                    op0=mybir.AluOpType.add, op1=mybir.AluOpType.max,
                )

                h2p = mpp2.tile([32, BS], F32, tag="h2")
                nc.tensor.matmul(
                    h2p, w2t[:], h1[:], start=True, stop=True
                )
                h2 = mlp.tile([32, BS], F16)
                nc.vector.tensor_scalar(
                    out=h2[:], in0=h2p[:], scalar1=b2_sb[:, :], scalar2=0.0,
                    op0=mybir.AluOpType.add, op1=mybir.AluOpType.max,
                )

                evp = mpp2.tile([1, BS], F32, tag="ev")
                nc.tensor.matmul(
                    evp, w3t[:], h2[:], start=True, stop=True
                )
                ev = mlp.tile([1, BS], F32)
                nc.vector.tensor_scalar(
                    out=ev[:], in0=evp[:], scalar1=b3_sb[:, :], scalar2=None,
                    op0=mybir.AluOpType.add,
                )
                evs = mlp.tile([1, BS], F32)
                nc.vector.tensor_mul(out=evs[:], in0=ev[:], in1=stm_sb[:])
                nc.sync.dma_start(out=out[:, :], in_=evs[:])

    _split_multi_waits(nc)
    return nc


_NC_CACHE: dict = {}


def _get_nc(**_ignored) -> bass.Bass:
    if "nc" not in _NC_CACHE:
        _NC_CACHE["nc"] = build_kernel()
    return _NC_CACHE["nc"]


def _kmajor(rows_f32: np.ndarray, ncols: int) -> np.ndarray:
    """[ncols, IN] fp32 -> [128, NKT, ncols] fp16, t[p, kt, c] = a[c, kt*128+p]."""
    return rows_f32.reshape(ncols, NKT, 128).transpose(2, 1, 0).astype(np.float16)


def make_in_maps(inputs: dict) -> list:
    """Shard full inputs into per-core input maps (cast + transpose + pack)."""
    wf = np.asarray(inputs["white_features"], dtype=np.float32)
    bf = np.asarray(inputs["black_features"], dtype=np.float32)
    stm = np.ascontiguousarray(inputs["side_to_move"], dtype=np.float32)
    wk = _kmajor(np.asarray(inputs["W_ft"], dtype=np.float32), H)  # [128,NKT,256]
    w1T = np.asarray(inputs["W1"], dtype=np.float32).T  # [512, 32]
    w1Ts = np.ascontiguousarray(
        w1T.reshape(4, 128, 32).transpose(1, 0, 2)
    ).reshape(128, 128).astype(np.float16)
    w2T = np.ascontiguousarray(
        np.asarray(inputs["W2"], dtype=np.float32).T).astype(np.float16)
    w3T = np.ascontiguousarray(
        np.asarray(inputs["W3"], dtype=np.float32).T).astype(np.float16)
    b1 = np.ascontiguousarray(inputs["b1"], dtype=np.float32).reshape(32, 1)
    b2 = np.ascontiguousarray(inputs["b2"], dtype=np.float32).reshape(32, 1)
    b3 = np.ascontiguousarray(inputs["b3"], dtype=np.float32).reshape(1, 1)
    maps = []
    for c in range(N_CORES):
        sl = slice(c * BS, (c + 1) * BS)
        packed = np.empty((128, NKT, PK), dtype=np.float16)
        packed[:, :, 0:BS] = _kmajor(wf[sl], BS)
        packed[:, :, BS:2 * BS] = _kmajor(bf[sl], BS)
        packed[:, :, 2 * BS:] = wk
        maps.append({
            "packed": packed,
            "side_to_move": stm[sl].reshape(1, BS),
            "W1Ts": w1Ts,
            "b1": b1,
            "W2T": w2T,
            "b2": b2,
            "W3T": w3T,
            "b3": b3,
        })
    return maps


def run(inputs: dict, trace: bool = False, **_ignored):
    """Run on all 8 cores; returns (full_output [4096,1] fp32, BassKernelResults)."""
    from concourse.bass_utils import run_bass_kernel_spmd

    nc = _get_nc()
    res = run_bass_kernel_spmd(
        nc, make_in_maps(inputs), core_ids=list(range(N_CORES)), trace=trace
    )
    full = np.concatenate(
        [res.results[c]["evaluation"].reshape(BS, 1) for c in range(N_CORES)],
        axis=0,
    ).astype(np.float32)
    return full, res


def kernel(**inputs) -> np.ndarray:
    return run(inputs, trace=False)[0]


if __name__ == "__main__":
    rng = np.random.default_rng(0)
    ins = {
        "white_features": rng.random((B, IN), dtype=np.float32),
        "black_features": rng.random((B, IN), dtype=np.float32),
        "side_to_move": np.ones((B,), dtype=np.float32),
        "W_ft": (0.1 * rng.standard_normal((H, IN))).astype(np.float32),
        "W1": (0.06 * rng.standard_normal((32, 2 * H))).astype(np.float32),
        "b1": np.zeros(32, np.float32),
        "W2": (0.17 * rng.standard_normal((32, 32))).astype(np.float32),
        "b2": np.zeros(32, np.float32),
        "W3": (0.24 * rng.standard_normal((1, 32))).astype(np.float32),
        "b3": np.zeros(1, np.float32),
    }
    out = kernel(**ins)
    # host reference
    whr = np.clip(ins["white_features"] @ ins["W_ft"].T, 0, 1)
    bhr = np.clip(ins["black_features"] @ ins["W_ft"].T, 0, 1)
    x = np.concatenate([whr, bhr], axis=1)
    x = np.maximum(x @ ins["W1"].T + ins["b1"], 0)
    x = np.maximum(x @ ins["W2"].T + ins["b2"], 0)
    ref = (x @ ins["W3"].T + ins["b3"]) * ins["side_to_move"][:, None]
    rel = np.linalg.norm(out - ref) / np.linalg.norm(ref)
    print("rel err:", rel)


# revision 11
# speedup vs baseline: 1.3412x; 1.0116x over previous
"""NNUE evaluation kernel for Trainium2 (8 NeuronCores, data-parallel batch).

reference math:
    wh = clip(white @ W_ft.T, 0, 1)        # [B, 256]
    bh = clip(black @ W_ft.T, 0, 1)        # [B, 256]
    x  = concat(wh, bh)                    # [B, 512]
    x  = relu(x @ W1.T + b1); x = relu(x @ W2.T + b2)
    ev = (x @ W3.T + b3) * stm[:, None]    # [B, 1]

Strategy: shard B=4096 across 8 cores (512 rows each), data-parallel, no
collectives. The kernel is HBM-bound, so everything on the feature-GEMM
path is cast to fp16 (e5m10) on the host: features are uniform [0,1] and
W_ft is N(0, 0.1), both comfortably inside fp16 range, and the PE
multiplies bf16/fp16 at the full 1 row/cycle rate with fp32 PSUM
accumulation (simulated end-to-end rel err ~7e-4 vs 2e-2 budget). That
halves the per-core HBM traffic from ~210 MB to ~105 MB (~276 us at the
~380 GB/s 16-engine DMA ceiling measured on this part).

The host pre-transposes everything into k-major partition-first layout
and PACKS white, black and W_ft.T into a single [128, 320, 1280] fp16
array per core (per k-tile: 512 white cols | 512 black cols | 256 W_ftT
cols). The kernel then needs NO on-chip transposes (the fp32 baseline
burned ~300 us of PE+DVE on matmul-with-identity transposes) and the
whole 105 MB streams as ONE run of large DMAs on one queue — 40 KB
contiguous per partition per slab — which keeps all 16 DMA engines at
line rate with no inter-queue arbitration. The first slabs are small
(4/4/8 k-tiles vs 16) so the PE starts ~5 us after launch instead of
waiting for a full 5 MB slab. out.T [h, b] accumulates in 4 PSUM banks
across all 320 k-tiles, the clip fuses into PSUM evacuation, and the
tiny MLP runs in transposed [features, batch] layout to the end.

This walrus build rejects instructions with >1 sync wait, so a post-pass
(_split_multi_waits) redistributes Tile-emitted waits onto single-wait
no-ops.
"""

import sys
import types

import numpy as np


def _inject_ntff_hook():
    """Register the axon NTFF profile hook if this image's antenv lacks it."""
    try:
        import antenv.axon_hooks  # noqa: F401
        return
    except ImportError:
        pass
    try:
        import trn_agent_boot.trn_boot as tb
        hook = tb._ntff_profile_via_ctypes("/opt/axon/libaxon_pjrt.so")
    except Exception:
        hook = None
    mod = types.ModuleType("antenv.axon_hooks")
    mod.get_axon_ntff_profile_hook = lambda: hook
    mod.set_axon_ntff_profile_hook = lambda h: None
    sys.modules["antenv.axon_hooks"] = mod


_inject_ntff_hook()

import concourse.bass as bass
import concourse.mybir as mybir
from concourse.tile import TileContext

N_CORES = 8
B = 4096
BS = B // N_CORES          # 512 batch rows per core
IN = 40960                 # feature count (contraction dim)
H = 256                    # hidden per perspective
NKT = IN // 128            # 320 k-tiles of 128
PK = 2 * BS + H            # packed columns per k-tile: white | black | W_ftT
# k-tiles per DMA slab. Small first slabs so the PE starts ~4 us after
# the stream opens instead of waiting on a full slab; small last slabs so
# the PE (which trails the DMA stream by ~one slab) has almost nothing
# left to chew once the last byte lands.
SLABS = [4, 4] + [8] * 38 + [4, 2, 2]
assert sum(SLABS) == NKT

F32 = mybir.dt.float32
F16 = mybir.dt.float16


def _split_multi_waits(nc: bass.Bass) -> None:
    """This walrus build rejects instructions carrying more than one sync
    wait. Split any such instruction: emit single-wait no-ops on the same
    engine immediately before it (same engine stream => same semantics)."""
    for f in nc.m.functions:
        for bb in f.blocks:
            new_insts = []
            changed = False
            for inst in bb.instructions:
                si = inst.sync_info
                waits = list(si.on_wait) if si is not None and si.on_wait else []
                if len(waits) > 1:
                    changed = True
                    for i, w in enumerate(waits[:-1]):
                        nop = mybir.InstNoOp(
                            name=f"{inst.name}-sw{i}", ins=[], outs=[]
                        )
                        nop.engine = inst.engine
                        nop.sync_info = mybir.SyncInfo(on_wait=[w], on_update=[])
                        nc.register_instruction(nop)
                        new_insts.append(nop)
                    inst.sync_info = mybir.SyncInfo(
                        on_wait=[waits[-1]],
                        on_update=list(si.on_update) if si.on_update else [],
                    )
                new_insts.append(inst)
            if changed:
                bb.instructions = new_insts


def build_kernel() -> bass.Bass:
    nc = bass.Bass()

    # packed[p, kt, :] = [white[b, kt*128+p] for b in 512] ++ [black ...]
    #                    ++ [W_ft[h, kt*128+p] for h in 256]
    packed = nc.dram_tensor("packed", [128, NKT, PK], F16, kind="ExternalInput")
    w1Ts = nc.dram_tensor("W1Ts", [128, 128], F16, kind="ExternalInput")
    b1 = nc.dram_tensor("b1", [32, 1], F32, kind="ExternalInput")
    w2T = nc.dram_tensor("W2T", [32, 32], F16, kind="ExternalInput")
    b2 = nc.dram_tensor("b2", [32, 1], F32, kind="ExternalInput")
    w3T = nc.dram_tensor("W3T", [32, 1], F16, kind="ExternalInput")
    b3 = nc.dram_tensor("b3", [1, 1], F32, kind="ExternalInput")
    stm = nc.dram_tensor("side_to_move", [1, BS], F32, kind="ExternalInput")
    out = nc.dram_tensor("evaluation", [1, BS], F32, kind="ExternalOutput")

    with TileContext(nc) as tc:
        with (
            tc.tile_pool(name="ot_psum", bufs=1, space="PSUM") as ot_pool,
            tc.tile_pool(name="mlp", bufs=1) as mlp,
        ):
            # out.T accumulators: [h-tile 128, b 512] x (2 sides x 2 h-tiles)
            ot = [
                ot_pool.tile([128, BS], F32, tag=f"ot{i}", name=f"ot{i}")
                for i in range(4)
            ]

            # ---- MLP weight prep: tiny DMAs, issued up-front on the
            # (otherwise idle) scalar queue so the tail never waits ----
            w1t = mlp.tile([128, 4, 32], F16)
            nc.scalar.dma_start(out=w1t[:], in_=w1Ts[:, :])
            w2t = mlp.tile([32, 32], F16)
            nc.scalar.dma_start(out=w2t[:], in_=w2T[:, :])
            w3t = mlp.tile([32, 1], F16)
            nc.scalar.dma_start(out=w3t[:], in_=w3T[:, :])
            b1_sb = mlp.tile([32, 1], F32)
            nc.scalar.dma_start(out=b1_sb[:], in_=b1[:, :])
            b2_sb = mlp.tile([32, 1], F32)
            nc.scalar.dma_start(out=b2_sb[:], in_=b2[:, :])
            b3_sb = mlp.tile([1, 1], F32)
            nc.scalar.dma_start(out=b3_sb[:], in_=b3[:, :])
            stm_sb = mlp.tile([1, BS], F32)
            nc.scalar.dma_start(out=stm_sb[:], in_=stm[:, :])

            # ---- main loop: feature-transformer GEMMs ----
            with (
                tc.tile_pool(name="ramp", bufs=1) as ramp_pool,
                tc.tile_pool(name="slab", bufs=6) as slab_pool,
            ):
                kt_base = 0
                for si, sk in enumerate(SLABS):
                    if sk < 8:
                        t = ramp_pool.tile([128, sk, PK], F16, tag=f"r{si}")
                    else:
                        t = slab_pool.tile([128, sk, PK], F16, tag="slab")
                    nc.sync.dma_start(
                        out=t[:], in_=packed[:, kt_base:kt_base + sk, :]
                    )
                    for kt in range(sk):
                        g = kt_base + kt
                        first = g == 0
                        last = g == NKT - 1
                        for side in range(2):
                            for h in range(2):
                                nc.tensor.matmul(
                                    ot[side * 2 + h],
                                    t[:, kt, 2 * BS + h * 128:2 * BS + (h + 1) * 128],
                                    t[:, kt, side * BS:(side + 1) * BS],
                                    start=first,
                                    stop=last,
                                )
                    kt_base += sk

            # ---- clip + MLP (transposed layout throughout) ----
            with tc.tile_pool(name="mlp2_psum", bufs=1, space="PSUM") as mpp2:
                xt = []
                for i in range(4):
                    t = mlp.tile([128, BS], F16, tag=f"xt{i}")
                    nc.vector.tensor_scalar(
                        out=t[:], in0=ot[i][:], scalar1=0.0, scalar2=1.0,
                        op0=mybir.AluOpType.max, op1=mybir.AluOpType.min,
                    )
                    xt.append(t)

                h1p = mpp2.tile([32, BS], F32, tag="h1")
                for kt in range(4):
                    nc.tensor.matmul(
                        h1p, w1t[:, kt, :], xt[kt][:],
                        start=kt == 0, stop=kt == 3,
                    )
                h1 = mlp.tile([32, BS], F16)
                nc.vector.tensor_scalar(
                    out=h1[:], in0=h1p[:], scalar1=b1_sb[:, :], scalar2=0.0,
                    op0=mybir.AluOpType.add, op1=mybir.AluOpType.max,
                )

                h2p = mpp2.tile([32, BS], F32, tag="h2")
                nc.tensor.matmul(
                    h2p, w2t[:], h1[:], start=True, stop=True
                )
                h2 = mlp.tile([32, BS], F16)
                nc.vector.tensor_scalar(
                    out=h2[:], in0=h2p[:], scalar1=b2_sb[:, :], scalar2=0.0,
                    op0=mybir.AluOpType.add, op1=mybir.AluOpType.max,
                )

                evp = mpp2.tile([1, BS], F32, tag="ev")
                nc.tensor.matmul(
                    evp, w3t[:], h2[:], start=True, stop=True
                )
                ev = mlp.tile([1, BS], F32)
                nc.vector.tensor_scalar(
                    out=ev[:], in0=evp[:], scalar1=b3_sb[:, :], scalar2=None,
                    op0=mybir.AluOpType.add,
                )
                evs = mlp.tile([1, BS], F32)
                nc.vector.tensor_mul(out=evs[:], in0=ev[:], in1=stm_sb[:])
                nc.sync.dma_start(out=out[:, :], in_=evs[:])

    _split_multi_waits(nc)
    return nc


_NC_CACHE: dict = {}


def _get_nc(**_ignored) -> bass.Bass:
    if "nc" not in _NC_CACHE:
        _NC_CACHE["nc"] = build_kernel()
    return _NC_CACHE["nc"]


def _kmajor(rows_f32: np.ndarray, ncols: int) -> np.ndarray:
    """[ncols, IN] fp32 -> [128, NKT, ncols] fp16, t[p, kt, c] = a[c, kt*128+p]."""
    return rows_f32.reshape(ncols, NKT, 128).transpose(2, 1, 0).astype(np.float16)


def make_in_maps(inputs: dict) -> list:
    """Shard full inputs into per-core input maps (cast + transpose + pack)."""
    wf = np.asarray(inputs["white_features"], dtype=np.float32)
    bf = np.asarray(inputs["black_features"], dtype=np.float32)
    stm = np.ascontiguousarray(inputs["side_to_move"], dtype=np.float32)
    wk = _kmajor(np.asarray(inputs["W_ft"], dtype=np.float32), H)  # [128,NKT,256]
    w1T = np.asarray(inputs["W1"], dtype=np.float32).T  # [512, 32]
    w1Ts = np.ascontiguousarray(
        w1T.reshape(4, 128, 32).transpose(1, 0, 2)
    ).reshape(128, 128).astype(np.float16)
    w2T = np.ascontiguousarray(
        np.asarray(inputs["W2"], dtype=np.float32).T).astype(np.float16)
    w3T = np.ascontiguousarray(
        np.asarray(inputs["W3"], dtype=np.float32).T).astype(np.float16)
    b1 = np.ascontiguousarray(inputs["b1"], dtype=np.float32).reshape(32, 1)
    b2 = np.ascontiguousarray(inputs["b2"], dtype=np.float32).reshape(32, 1)
    b3 = np.ascontiguousarray(inputs["b3"], dtype=np.float32).reshape(1, 1)
    maps = []
    for c in range(N_CORES):
        sl = slice(c * BS, (c + 1) * BS)
        packed = np.empty((128, NKT, PK), dtype=np.float16)
        packed[:, :, 0:BS] = _kmajor(wf[sl], BS)
        packed[:, :, BS:2 * BS] = _kmajor(bf[sl], BS)
        packed[:, :, 2 * BS:] = wk
        maps.append({
            "packed": packed,
            "side_to_move": stm[sl].reshape(1, BS),
            "W1Ts": w1Ts,
            "b1": b1,
            "W2T": w2T,
            "b2": b2,
            "W3T": w3T,
            "b3": b3,
        })
    return maps


def run(inputs: dict, trace: bool = False, **_ignored):
    """Run on all 8 cores; returns (full_output [4096,1] fp32, BassKernelResults)."""
    from concourse.bass_utils import run_bass_kernel_spmd

    nc = _get_nc()
    res = run_bass_kernel_spmd(
        nc, make_in_maps(inputs), core_ids=list(range(N_CORES)), trace=trace
    )
    full = np.concatenate(
        [res.results[c]["evaluation"].reshape(BS, 1) for c in range(N_CORES)],
        axis=0,
    ).astype(np.float32)
    return full, res


def kernel(**inputs) -> np.ndarray:
    return run(inputs, trace=False)[0]


if __name__ == "__main__":
    rng = np.random.default_rng(0)
    ins = {
        "white_features": rng.random((B, IN), dtype=np.float32),
        "black_features": rng.random((B, IN), dtype=np.float32),
        "side_to_move": np.ones((B,), dtype=np.float32),
        "W_ft": (0.1 * rng.standard_normal((H, IN))).astype(np.float32),
        "W1": (0.06 * rng.standard_normal((32, 2 * H))).astype(np.float32),
        "b1": np.zeros(32, np.float32),
        "W2": (0.17 * rng.standard_normal((32, 32))).astype(np.float32),
        "b2": np.zeros(32, np.float32),
        "W3": (0.24 * rng.standard_normal((1, 32))).astype(np.float32),
        "b3": np.zeros(1, np.float32),
    }
    out = kernel(**ins)
    # host reference
    whr = np.clip(ins["white_features"] @ ins["W_ft"].T, 0, 1)
    bhr = np.clip(ins["black_features"] @ ins["W_ft"].T, 0, 1)
    x = np.concatenate([whr, bhr], axis=1)
    x = np.maximum(x @ ins["W1"].T + ins["b1"], 0)
    x = np.maximum(x @ ins["W2"].T + ins["b2"], 0)
    ref = (x @ ins["W3"].T + ins["b3"]) * ins["side_to_move"][:, None]
    rel = np.linalg.norm(out - ref) / np.linalg.norm(ref)
    print("rel err:", rel)


# revision 16
# speedup vs baseline: 1.4106x; 1.0518x over previous
"""NNUE evaluation kernel for Trainium2 (8 NeuronCores, data-parallel batch).

reference math:
    wh = clip(white @ W_ft.T, 0, 1)        # [B, 256]
    bh = clip(black @ W_ft.T, 0, 1)        # [B, 256]
    x  = concat(wh, bh)                    # [B, 512]
    x  = relu(x @ W1.T + b1); x = relu(x @ W2.T + b2)
    ev = (x @ W3.T + b3) * stm[:, None]    # [B, 1]

Strategy: shard B=4096 across 8 cores (512 rows each), data-parallel, no
collectives. The kernel is HBM-bound, so everything on the feature-GEMM
path is cast to fp16 (e5m10) on the host: features are uniform [0,1] and
W_ft is N(0, 0.1), both comfortably inside fp16 range, and the PE
multiplies bf16/fp16 at the full 1 row/cycle rate with fp32 PSUM
accumulation (simulated end-to-end rel err ~7e-4 vs 2e-2 budget). That
halves the per-core HBM traffic from ~210 MB to ~105 MB (~276 us at the
~380 GB/s 16-engine DMA ceiling measured on this part).

The host pre-transposes everything into k-major partition-first layout
and PACKS white, black and W_ft.T into a single [128, 320, 1280] fp16
array per core (per k-tile: 512 white cols | 512 black cols | 256 W_ftT
cols). The kernel then needs NO on-chip transposes (the fp32 baseline
burned ~300 us of PE+DVE on matmul-with-identity transposes) and the
whole 105 MB streams as ONE run of large DMAs on one queue — 40 KB
contiguous per partition per slab — which keeps all 16 DMA engines at
line rate with no inter-queue arbitration. The first slabs are small
(4/4/8 k-tiles vs 16) so the PE starts ~5 us after launch instead of
waiting for a full 5 MB slab. out.T [h, b] accumulates in 4 PSUM banks
across all 320 k-tiles, the clip fuses into PSUM evacuation, and the
tiny MLP runs in transposed [features, batch] layout to the end.

This walrus build rejects instructions with >1 sync wait, so a post-pass
(_split_multi_waits) redistributes Tile-emitted waits onto single-wait
no-ops.
"""

import sys
import types

import numpy as np


def _inject_ntff_hook():
    """Register the axon NTFF profile hook if this image's antenv lacks it."""
    try:
        import antenv.axon_hooks  # noqa: F401
        return
    except ImportError:
        pass
    try:
        import trn_agent_boot.trn_boot as tb
        hook = tb._ntff_profile_via_ctypes("/opt/axon/libaxon_pjrt.so")
    except Exception:
        hook = None
    mod = types.ModuleType("antenv.axon_hooks")
    mod.get_axon_ntff_profile_hook = lambda: hook
    mod.set_axon_ntff_profile_hook = lambda h: None
    sys.modules["antenv.axon_hooks"] = mod


_inject_ntff_hook()

import concourse.bass as bass
import concourse.mybir as mybir
from concourse.tile import TileContext

N_CORES = 8
B = 4096
BS = B // N_CORES          # 512 batch rows per core
IN = 40960                 # feature count (contraction dim)
H = 256                    # hidden per perspective
NKT = IN // 128            # 320 k-tiles of 128
PK = 2 * BS + H            # packed columns per k-tile: white | black | W_ftT
# k-tiles per DMA slab. Small first slabs so the PE starts ~4 us after
# the stream opens instead of waiting on a full slab; small last slabs so
# the PE (which trails the DMA stream by ~one slab) has almost nothing
# left to chew once the last byte lands.
SLABS = [4, 4] + [8] * 38 + [4, 2, 2]
assert sum(SLABS) == NKT

F32 = mybir.dt.float32
F16 = mybir.dt.float16


def _split_multi_waits(nc: bass.Bass) -> None:
    """This walrus build rejects instructions carrying more than one sync
    wait. Split any such instruction: emit single-wait no-ops on the same
    engine immediately before it (same engine stream => same semantics)."""
    for f in nc.m.functions:
        for bb in f.blocks:
            new_insts = []
            changed = False
            for inst in bb.instructions:
                si = inst.sync_info
                waits = list(si.on_wait) if si is not None and si.on_wait else []
                if len(waits) > 1:
                    changed = True
                    for i, w in enumerate(waits[:-1]):
                        nop = mybir.InstNoOp(
                            name=f"{inst.name}-sw{i}", ins=[], outs=[]
                        )
                        nop.engine = inst.engine
                        nop.sync_info = mybir.SyncInfo(on_wait=[w], on_update=[])
                        nc.register_instruction(nop)
                        new_insts.append(nop)
                    inst.sync_info = mybir.SyncInfo(
                        on_wait=[waits[-1]],
                        on_update=list(si.on_update) if si.on_update else [],
                    )
                new_insts.append(inst)
            if changed:
                bb.instructions = new_insts


def build_kernel() -> bass.Bass:
    nc = bass.Bass()

    # packed[p, kt, :] = [white[b, kt*128+p] for b in 512] ++ [black ...]
    #                    ++ [W_ft[h, kt*128+p] for h in 256]
    packed = nc.dram_tensor("packed", [128, NKT, PK], F16, kind="ExternalInput")
    # MLP weights packed into one fp16 tensor: cols 0:128 = W1Ts (4 k-tiles
    # x 32), [0:32, 128:160] = W2T, [0:32, 160] = W3T.
    mlp16 = nc.dram_tensor("mlp16", [128, 161], F16, kind="ExternalInput")
    # fp32 side-channel: col 0 = b1, col 1 = b2, [0, 2] = b3,
    # [0, 3:515] = side_to_move.
    mlp32 = nc.dram_tensor("mlp32", [32, 3 + BS], F32, kind="ExternalInput")
    out = nc.dram_tensor("evaluation", [1, BS], F32, kind="ExternalOutput")

    with TileContext(nc) as tc:
        with (
            tc.tile_pool(name="ot_psum", bufs=1, space="PSUM") as ot_pool,
            tc.tile_pool(name="mlp", bufs=1) as mlp,
        ):
            # out.T accumulators: [h-tile 128, b 512] x (2 sides x 2 h-tiles)
            ot = [
                ot_pool.tile([128, BS], F32, tag=f"ot{i}", name=f"ot{i}")
                for i in range(4)
            ]

            # ---- MLP weight prep: two tiny DMAs, issued up-front on the
            # scalar queue so the tail never waits ----
            m16 = mlp.tile([128, 161], F16)
            nc.scalar.dma_start(out=m16[:], in_=mlp16[:, :])
            m32 = mlp.tile([32, 3 + BS], F32)
            nc.scalar.dma_start(out=m32[:], in_=mlp32[:, :])
            w2t = m16[0:32, 128:160]
            w3t = m16[0:32, 160:161]
            b1_sb = m32[:, 0:1]
            b2_sb = m32[:, 1:2]
            b3_sb = m32[0:1, 2:3]
            stm_sb = m32[0:1, 3:3 + BS]

            # ---- main loop: feature-transformer GEMMs ----
            with (
                tc.tile_pool(name="ramp", bufs=1) as ramp_pool,
                tc.tile_pool(name="slab", bufs=6) as slab_pool,
            ):
                kt_base = 0
                for si, sk in enumerate(SLABS):
                    if sk < 8:
                        t = ramp_pool.tile([128, sk, PK], F16, tag=f"r{si}")
                    else:
                        t = slab_pool.tile([128, sk, PK], F16, tag="slab")
                    # alternate between the two hardware DGE queues so the
                    # stream is robust to per-queue arbitration stalls
                    eng = nc.sync if si % 2 == 0 else nc.scalar
                    eng.dma_start(
                        out=t[:], in_=packed[:, kt_base:kt_base + sk, :]
                    )
                    for kt in range(sk):
                        g = kt_base + kt
                        first = g == 0
                        last = g == NKT - 1
                        for side in range(2):
                            for h in range(2):
                                nc.tensor.matmul(
                                    ot[side * 2 + h],
                                    t[:, kt, 2 * BS + h * 128:2 * BS + (h + 1) * 128],
                                    t[:, kt, side * BS:(side + 1) * BS],
                                    start=first,
                                    stop=last,
                                )
                    kt_base += sk

            # ---- clip + MLP (transposed layout throughout; PSUM tiles
            # come from the same pool as the accumulators) ----
            xt = []
            for i in range(4):
                t = mlp.tile([128, BS], F16, tag=f"xt{i}")
                nc.vector.tensor_scalar(
                    out=t[:], in0=ot[i][:], scalar1=0.0, scalar2=1.0,
                    op0=mybir.AluOpType.max, op1=mybir.AluOpType.min,
                )
                xt.append(t)

            h1p = ot_pool.tile([32, BS], F32, tag="h1")
            for kt in range(4):
                nc.tensor.matmul(
                    h1p, m16[:, kt * 32:(kt + 1) * 32], xt[kt][:],
                    start=kt == 0, stop=kt == 3,
                )
            h1 = mlp.tile([32, BS], F16)
            nc.vector.tensor_scalar(
                out=h1[:], in0=h1p[:], scalar1=b1_sb, scalar2=0.0,
                op0=mybir.AluOpType.add, op1=mybir.AluOpType.max,
            )

            h2p = ot_pool.tile([32, BS], F32, tag="h2")
            nc.tensor.matmul(
                h2p, w2t, h1[:], start=True, stop=True
            )
            h2 = mlp.tile([32, BS], F16)
            nc.vector.tensor_scalar(
                out=h2[:], in0=h2p[:], scalar1=b2_sb, scalar2=0.0,
                op0=mybir.AluOpType.add, op1=mybir.AluOpType.max,
            )

            evp = ot_pool.tile([1, BS], F32, tag="ev")
            nc.tensor.matmul(
                evp, w3t, h2[:], start=True, stop=True
            )
            ev = mlp.tile([1, BS], F32)
            nc.vector.tensor_scalar(
                out=ev[:], in0=evp[:], scalar1=b3_sb, scalar2=None,
                op0=mybir.AluOpType.add,
            )
            evs = mlp.tile([1, BS], F32)
            nc.vector.tensor_mul(out=evs[:], in0=ev[:], in1=stm_sb)
            nc.sync.dma_start(out=out[:, :], in_=evs[:])

    _split_multi_waits(nc)
    return nc


_NC_CACHE: dict = {}


def _get_nc(**_ignored) -> bass.Bass:
    if "nc" not in _NC_CACHE:
        _NC_CACHE["nc"] = build_kernel()
    return _NC_CACHE["nc"]


def _kmajor(rows_f32: np.ndarray, ncols: int) -> np.ndarray:
    """[ncols, IN] fp32 -> [128, NKT, ncols] fp16, t[p, kt, c] = a[c, kt*128+p]."""
    return rows_f32.reshape(ncols, NKT, 128).transpose(2, 1, 0).astype(np.float16)


def make_in_maps(inputs: dict) -> list:
    """Shard full inputs into per-core input maps (cast + transpose + pack)."""
    wf = np.asarray(inputs["white_features"], dtype=np.float32)
    bf = np.asarray(inputs["black_features"], dtype=np.float32)
    stm = np.ascontiguousarray(inputs["side_to_move"], dtype=np.float32)
    wk = _kmajor(np.asarray(inputs["W_ft"], dtype=np.float32), H)  # [128,NKT,256]
    w1T = np.asarray(inputs["W1"], dtype=np.float32).T  # [512, 32]
    w1Ts = np.ascontiguousarray(
        w1T.reshape(4, 128, 32).transpose(1, 0, 2)
    ).reshape(128, 128).astype(np.float16)
    mlp16 = np.zeros((128, 161), dtype=np.float16)
    mlp16[:, 0:128] = w1Ts
    mlp16[0:32, 128:160] = np.asarray(inputs["W2"], dtype=np.float32).T
    mlp16[0:32, 160] = np.asarray(inputs["W3"], dtype=np.float32).reshape(32)
    mlp32_base = np.zeros((32, 3 + BS), dtype=np.float32)
    mlp32_base[:, 0] = np.asarray(inputs["b1"], dtype=np.float32).reshape(32)
    mlp32_base[:, 1] = np.asarray(inputs["b2"], dtype=np.float32).reshape(32)
    mlp32_base[0, 2] = np.asarray(inputs["b3"], dtype=np.float32).reshape(1)[0]
    maps = []
    for c in range(N_CORES):
        sl = slice(c * BS, (c + 1) * BS)
        packed = np.empty((128, NKT, PK), dtype=np.float16)
        packed[:, :, 0:BS] = _kmajor(wf[sl], BS)
        packed[:, :, BS:2 * BS] = _kmajor(bf[sl], BS)
        packed[:, :, 2 * BS:] = wk
        mlp32 = mlp32_base.copy()
        mlp32[0, 3:] = stm[sl]
        maps.append({
            "packed": packed,
            "mlp16": mlp16,
            "mlp32": mlp32,
        })
    return maps


def run(inputs: dict, trace: bool = False, **_ignored):
    """Run on all 8 cores; returns (full_output [4096,1] fp32, BassKernelResults)."""
    from concourse.bass_utils import run_bass_kernel_spmd

    nc = _get_nc()
    res = run_bass_kernel_spmd(
        nc, make_in_maps(inputs), core_ids=list(range(N_CORES)), trace=trace
    )
    full = np.concatenate(
        [res.results[c]["evaluation"].reshape(BS, 1) for c in range(N_CORES)],
        axis=0,
    ).astype(np.float32)
    return full, res


def kernel(**inputs) -> np.ndarray:
    return run(inputs, trace=False)[0]


if __name__ == "__main__":
    rng = np.random.default_rng(0)
    ins = {
        "white_features": rng.random((B, IN), dtype=np.float32),
        "black_features": rng.random((B, IN), dtype=np.float32),
        "side_to_move": np.ones((B,), dtype=np.float32),
        "W_ft": (0.1 * rng.standard_normal((H, IN))).astype(np.float32),
        "W1": (0.06 * rng.standard_normal((32, 2 * H))).astype(np.float32),
        "b1": np.zeros(32, np.float32),
        "W2": (0.17 * rng.standard_normal((32, 32))).astype(np.float32),
        "b2": np.zeros(32, np.float32),
        "W3": (0.24 * rng.standard_normal((1, 32))).astype(np.float32),
        "b3": np.zeros(1, np.float32),
    }
    out = kernel(**ins)
    # host reference
    whr = np.clip(ins["white_features"] @ ins["W_ft"].T, 0, 1)
    bhr = np.clip(ins["black_features"] @ ins["W_ft"].T, 0, 1)
    x = np.concatenate([whr, bhr], axis=1)
    x = np.maximum(x @ ins["W1"].T + ins["b1"], 0)
    x = np.maximum(x @ ins["W2"].T + ins["b2"], 0)
    ref = (x @ ins["W3"].T + ins["b3"]) * ins["side_to_move"][:, None]
    rel = np.linalg.norm(out - ref) / np.linalg.norm(ref)
    print("rel err:", rel)
